# revision 7
# baseline (speedup 1.0000x reference)
"""Bass/Tile TRN2 kernel for nn_DiagonalLSTM (v3).

Data-parallel over batch: 16 batch elements across 8 cores -> 2 per core.
Per core, two independent pipelines ("groups", one per batch element)
run the 128-step LSTM scan.  The serial per-step dependency chain - not
engine throughput - bounds the runtime, so the design minimizes it:

  sh(t-1) -> taps-mms -> sigmoid(gates) -> cgs4 -> u4 -> c-update
          -> sigmoid(2c) -> sh(t)

- PSUM is initialized by a dependency-free zero matmul, and the x-term
  matmuls (valid rows only, stride-127 diagonal reads of natural-layout
  bf16 x) depend only on the input DMA: both run ahead; only the two
  bf16 state-tap matmuls (~53ns each) sit on the chain.
- The prev-tap (row shift by one) is the same ring slot read one column
  earlier; column 0 of each slot is a permanent zero pad.  No shifted
  copy, no h-half boundary handoff (rows are unsplit).
- All matmul operands live at partition base 0: members of one PSUM
  accumulation group must share a PE row band (mixed tile_positions
  crash the device).
- One act table for everything: the sigmoid_and_others set holds both
  Sigmoid and Tanh, so the gate sigmoid and tch = tanh(0.5*CH4) =
  tanh(c) share a table (no reloads).  c-state kept as CH4 = 2c;
  candidate gate via cgs4 = 4*sigmoid(2g)-2 = 2*tanh(g) with one
  dual-scalar tensor_scalar; the ring stores full h = tch * sig_o.
- Engine placement: cgs4/u4/add are one queue-contiguous DVE block per
  group (group-major: no cross-group interleave on the chain; the
  realign is forced by the verifier's equal-base rule for 2-input DVE
  ops); t1 and the final h-multiply on Pool (plain tensor_tensor is
  Pool-legal; stt is not); sigmas/tanh on ACT.  bf16 cell tiles enable
  the DVE 2x perf mode.
- The whole cell is bf16 except biases; measured rel err 9.7e-3 on HW
  (gate 2e-2, deterministic inputs).
- All weight prep (transposes, x2 scalings, bias fusion) happens on the
  host; device preamble is DMA + tiny memsets.  Output is a raw ring
  dump (contiguous SBUF DMA), decoded on the host.
- timing=True builds declare all data tensors Internal so repeated
  calls move no host bytes (used by test.py's interleaved differencing).
"""

import sys

sys.path.insert(0, "/opt/trn_rl_repo")

from contextlib import ExitStack

import numpy as np

import concourse.bass as bass
import concourse.tile as tile
from concourse import bacc, mybir

F32 = mybir.dt.float32
BF16 = mybir.dt.bfloat16
AF = mybir.ActivationFunctionType
ALU = mybir.AluOpType

N_CORES = 8
B = 2  # batch per core (= groups)
CIN = 32
H = 128  # rows
T = 128  # scan steps
BO = 32
G4 = 4 * BO  # gate channels, partition order (o, f, i, gg)
RS = 130  # ring slot columns: [pad, 128 rows, spare]
R = 16  # ring depth (slots)
CHUNK = 8  # output DMA chunk (steps); divides T, <= R/2
LOOSE_U = False  # the HW birverifier (NCC_IBIR297) demands equal base
# partitions for 2-input SBUF DVE ops, so u needs the cgs realign first
CELLDT = BF16  # cell-state dtype: bf16 enables the DVE 2x perf mode


def _build_module(reps=1, t_steps=None, no_out=False, timing=False,
                  no_x=False, no_tail=False, no_cell=False,
                  pool_add=False, t1_dve=True, h_dve=True):
    TS = T if t_steps is None else t_steps
    nc = bacc.Bacc(
        "TRN2",
        target_bir_lowering=False,
        debug=False,
        num_devices=N_CORES,
    )

    # timing builds take no external data (uninitialized internal DRAM;
    # instruction stream and therefore timing are identical) so repeated
    # calls move no host bytes
    ki = "Internal" if timing else "ExternalInput"
    ko = "Internal" if timing else "ExternalOutput"
    # natural-layout x, bf16, all at partitions 0-31 (cin), 4 column
    # blocks of 64*T: (g0 rows 0-63, g0 rows 64-127, g1 lo, g1 hi).
    # Matmuls may only mix within one PE row band: every matmul operand
    # lives at partition base 0 (mixed tile_positions in one PSUM
    # accumulation group crash the device).
    xc_d = nc.dram_tensor("xc", [CIN, 4 * 64 * T], BF16, kind=ki)
    wcur_d = nc.dram_tensor("wcur", [BO, G4], BF16, kind=ki)
    wprev_d = nc.dram_tensor("wprev", [BO, G4], BF16, kind=ki)
    w2t_d = nc.dram_tensor("w2t", [CIN, G4], BF16, kind=ki)
    bias_d = nc.dram_tensor("bias", [G4], F32, kind=ki)
    scale_d = nc.dram_tensor("scale", [G4], F32, kind=ki)
    # raw ring dump: [group, chunk, 32 gates, CHUNK slots * RS cols]
    hs_d = nc.dram_tensor(
        "hs", [B, TS // CHUNK, BO, CHUNK * RS], BF16, kind=ko
    )
    tiny_d = (
        nc.dram_tensor("tiny", [1, 4], F32, kind="ExternalOutput")
        if timing else None
    )

    with ExitStack() as ctx:
        tc = ctx.enter_context(tile.TileContext(nc))
        const = ctx.enter_context(tc.tile_pool(name="const", bufs=1))
        psum = ctx.enter_context(tc.tile_pool(name="psum", bufs=2, space="PSUM"))
        sig_p = ctx.enter_context(tc.tile_pool(name="sig", bufs=2))
        tmp_p = ctx.enter_context(tc.tile_pool(name="tmp", bufs=2))

        # ---- persistent tiles ----
        xc = const.tile([CIN, 4 * 64 * T], BF16, tag="xc")
        zq = const.tile([BO, H], BF16, tag="zq")  # zero matmul rhs
        # ring: partitions 0-31 = hA (h(r) at slot col 1+r), partitions
        # 32-63 = hB (h(r-1) at col 1+r, i.e. h(c-2) at col c); group g at
        # col offset g * R * RS.  The two blocks let one contraction-64
        # matmul compute both state taps (matmul cost is free-size only).
        ring = const.tile([2 * BO, B * R * RS], BF16, tag="ring")
        wtap = const.tile([2 * BO, G4], BF16, tag="wtap")
        wprev0 = const.tile([BO, G4], BF16, tag="wprev0")  # base-0 copy
        w2t4 = const.tile([CIN, G4], BF16, tag="w2t4")
        bias = const.tile([G4, 1], F32, tag="bias")
        scale = const.tile([G4, 1], F32, tag="scale")
        zb = const.tile([2 * BO, 1], F32, tag="zb")  # zero bias (base-32)
        # c-state (CH4 = 2c) per group at partition base 32
        ch = {g: const.tile([2 * BO, H], CELLDT, tag=f"ch{g}",
                            name=f"ch{g}")
              for g in range(B)}

        # ---- preamble ----
        nc.sync.dma_start(out=wtap[0:BO, :], in_=wcur_d.ap())
        nc.sync.dma_start(out=wtap[BO : 2 * BO, :], in_=wprev_d.ap())
        nc.sync.dma_start(out=wprev0[:, :], in_=wprev_d.ap())
        nc.sync.dma_start(out=w2t4[:, :], in_=w2t_d.ap())
        nc.sync.dma_start(out=bias[:, :], in_=bias_d.ap()[:, None])
        nc.sync.dma_start(out=scale[:, :], in_=scale_d.ap()[:, None])
        # x in 16-row chunks, interleaved across blocks so every block's
        # early rows land first (the SP sequencer issues DMAs serially at
        # ~565ns each; issue order gates scan startup)
        for q in range(4):
            for blk in range(4):
                c0 = blk * 64 * T + q * 16 * T
                c1 = c0 + 16 * T
                nc.sync.dma_start(out=xc[:, c0:c1], in_=xc_d.ap()[:, c0:c1])

        # slot R-1 (read by step 0's taps) first, then the rest
        for g in range(B):
            o = g * R * RS
            nc.vector.memset(ring[:, o + (R - 1) * RS : o + R * RS], 0.0)
        for g in range(B):
            o = g * R * RS
            nc.vector.memset(ring[:, o : o + (R - 1) * RS], 0.0)
        nc.vector.memset(zb[:, :], 0.0)
        nc.vector.memset(zq[:, :], 0.0)
        for g in range(B):
            nc.vector.memset(ch[g][:, :], 0.0)

        rv = ring[:, :].rearrange("p (g s c) -> p g s c", g=B, s=R)
        xv = xc[:, :].rearrange("p (b c) -> p b c", b=4)  # 4 column blocks

        # ---- the scan ----
        import contextlib

        rep_ctx = tc.For_i(0, reps, 1) if reps > 1 else contextlib.nullcontext()
        with rep_ctx:
            for t in range(TS):
                sp = (t - 1) % R
                sl = t % R

                def mm_phase(g):
                    gp = psum.tile([G4, H], F32, tag=f"g{g}", name=f"g{g}")
                    # PSUM init: zero matmul with no data dependencies
                    nc.tensor.matmul(
                        gp[:, :], zq[:, :], zq[:, :],
                        start=True, stop=False,
                    )
                    # x-term: valid rows only (diagonal stride-127 reads);
                    # off the serial chain (depends only on the input DMA)
                    if not no_x:
                        nlo = min(t + 1, 64)
                        nc.tensor.matmul(
                            gp[:, 0:nlo], w2t4[:, :],
                            xv[:, 2 * g, t : t + 127 * (nlo - 1) + 1 : 127],
                            start=False, stop=False,
                        )
                        if t >= 64:
                            nhi = t - 64 + 1
                            d0 = t - 64
                            nc.tensor.matmul(
                                gp[:, 64 : 64 + nhi], w2t4[:, :],
                                xv[:, 2 * g + 1,
                                   d0 : d0 + 127 * (nhi - 1) + 1 : 127],
                                start=False, stop=False,
                            )
                    # state taps (the only mms on the serial chain)
                    nc.tensor.matmul(
                        gp[:, :], wtap[0:BO, :],
                        rv[0:BO, g, sp, 1 : 1 + H],
                        start=False, stop=False,
                    )
                    nc.tensor.matmul(
                        gp[:, :], wprev0[:, :],
                        rv[0:BO, g, sp, 0:H],
                        start=False, stop=True,
                    )
                    return gp

                gps = [mm_phase(g) for g in range(B)]

                sg, w, t1, tch = {}, {}, {}, {}
                for g in range(B):
                    sg[g] = sig_p.tile([G4, H], CELLDT, tag=f"sg{g}",
                                       name=f"sg{g}")
                    nc.scalar.activation(
                        sg[g][:, :], gps[g][:, :], AF.Sigmoid,
                        bias=bias[:, :], scale=scale[:, :],
                    )
                if no_cell:
                    for g in range(B):
                        # timing ablation: ring write straight from sg
                        nc.vector.scalar_tensor_tensor(
                            rv[0:BO, g, sl, 1 : 1 + H], sg[g][0:BO, :],
                            0.5, sg[g][0:BO, :], ALU.subtract, ALU.mult,
                        )
                    continue
                if not t1_dve:
                    for g in range(B):
                        # t1 = sig_f * CH4 (base-32-preserving -> Pool)
                        t1[g] = tmp_p.tile([2 * BO, H], CELLDT, tag=f"t1{g}",
                                           name=f"t1{g}")[BO : 2 * BO, :]
                        nc.gpsimd.tensor_tensor(
                            t1[g], sg[g][BO : 2 * BO, :],
                            ch[g][BO : 2 * BO, :], ALU.mult,
                        )
                # DVE trio per group, group-major so the other group's ops
                # don't interleave into this group's chain
                for g in range(B):
                    # cgs4 = 4*sigmoid(2g_gg) - 2 = 2*tanh(g_gg),
                    # realigned from quadrant 96 to 64
                    w[g] = tmp_p.tile([3 * BO, H], CELLDT, tag=f"w{g}",
                                      name=f"w{g}")
                    nc.vector.tensor_scalar(
                        w[g][2 * BO : 3 * BO, :], sg[g][3 * BO : 4 * BO, :],
                        4.0, 2.0, ALU.mult, ALU.subtract,
                    )
                    if t1_dve:
                        # t1 = sig_f * CH4 second in the block: fills the
                        # cgs->u4 RAW pipeline-drain bubble
                        t1[g] = tmp_p.tile([2 * BO, H], CELLDT, tag=f"t1{g}",
                                           name=f"t1{g}")[BO : 2 * BO, :]
                        nc.vector.tensor_tensor(
                            t1[g], sg[g][BO : 2 * BO, :],
                            ch[g][BO : 2 * BO, :], ALU.mult,
                        )
                    # u4 = cgs4 * sig_i = 2*i*gg -> base 32
                    nc.vector.tensor_tensor(
                        w[g][BO : 2 * BO, :], w[g][2 * BO : 3 * BO, :],
                        sg[g][2 * BO : 3 * BO, :], ALU.mult,
                    )
                    # CH4 = u4 + t1 (queue-contiguous: no sem hop on chain)
                    eng_add = nc.gpsimd if pool_add else nc.vector
                    eng_add.tensor_tensor(
                        ch[g][BO : 2 * BO, :], w[g][BO : 2 * BO, :],
                        t1[g], ALU.add,
                    )
                if no_tail:
                    for g in range(B):
                        # timing ablation: ring write from CH4, no sigma4
                        nc.vector.scalar_tensor_tensor(
                            rv[0:BO, g, sl, 1 : 1 + H],
                            ch[g][BO : 2 * BO, :], 0.5, t1[g],
                            ALU.subtract, ALU.mult,
                        )
                    continue
                for g in range(B):
                    # tch = tanh(0.5*CH4) = tanh(c) (same act table as
                    # Sigmoid: sigmoid_and_others has both -> no reload)
                    tch[g] = tmp_p.tile([BO, H], CELLDT, tag=f"tch{g}",
                                        name=f"tch{g}")
                    nc.scalar.activation(
                        tch[g][:, :], ch[g][BO : 2 * BO, :], AF.Tanh,
                        bias=zb[BO : 2 * BO, :], scale=0.5,
                    )
                for g in range(B):
                    # sh = tch * sig_o = o*tanh(c) = h -> ring hA
                    eng_h = nc.vector if h_dve else nc.gpsimd
                    eng_h.tensor_tensor(
                        rv[0:BO, g, sl, 1 : 1 + H], tch[g][:, :],
                        sg[g][0:BO, :], ALU.mult,
                    )

                if t % CHUNK == CHUNK - 1 and not no_out:
                    # contiguous SBUF source (1 descriptor per partition)
                    c0 = t - CHUNK + 1
                    s0 = c0 % R
                    for g in range(B):
                        o0 = (g * R + s0) * RS
                        nc.sync.dma_start(
                            out=hs_d.ap()[g, t // CHUNK, :, :],
                            in_=ring[0:BO, o0 : o0 + CHUNK * RS],
                        )

        if timing:
            # tiny real output so the bass_exec call cannot be elided
            tt = const.tile([1, 4], F32, tag="tt")
            nc.vector.tensor_copy(tt[:, :], ring[0:1, 0:4])
            nc.sync.dma_start(out=tiny_d.ap(), in_=tt[:, :])

    nc.compile()
    return nc


_NC_CACHE = {}


def _get_module(**kw):
    key = tuple(sorted(kw.items()))
    if key not in _NC_CACHE:
        _NC_CACHE[key] = _build_module(**kw)
    return _NC_CACHE[key]


def _prep_weights(W2, b2, W1, b1):
    import ml_dtypes

    W2 = np.asarray(W2, np.float32)
    W1 = np.asarray(W1, np.float32)
    b = np.asarray(b1, np.float32) + np.asarray(b2, np.float32)
    bias = b.copy()
    bias[3 * BO :] *= 2.0  # gg rows: sigmoid(2z) trick needs doubled bias
    scale = np.ones(G4, np.float32)
    scale[3 * BO :] = 2.0
    bf = ml_dtypes.bfloat16
    wcur = np.ascontiguousarray(W1[:, :, 1].T).astype(bf)
    wprev = np.ascontiguousarray(W1[:, :, 0].T).astype(bf)
    w2t = np.ascontiguousarray(W2.T).astype(bf)
    return wcur, wprev, w2t, bias, scale


def _prep_canvas(x):
    """Per-core natural-layout x [CIN, 4*64*T] bf16, partitions 0-31,
    column blocks (g0 rows 0-63, g0 rows 64-127, g1 lo, g1 hi)."""
    import ml_dtypes

    nb, _, _, _ = x.shape  # (16, CIN, H, T)
    out = np.empty((nb // B, CIN, 4 * 64 * T), np.float32)
    for k in range(nb // B):
        for g in range(B):
            xb = x[B * k + g]  # (CIN, H, T)
            o = 2 * g * 64 * T
            out[k, :, o : o + 64 * T] = xb[:, 0:64, :].reshape(CIN, -1)
            out[k, :, o + 64 * T : o + 2 * 64 * T] = (
                xb[:, 64:128, :].reshape(CIN, -1)
            )
    return out.astype(ml_dtypes.bfloat16)


def kernel(x, W2, b2, W1, b1):
    from concourse.bass_utils import run_bass_kernel_spmd

    nc = _get_module()
    x = np.ascontiguousarray(x, dtype=np.float32)
    wcur, wprev, w2t, bias, scale = _prep_weights(W2, b2, W1, b1)
    xcs = _prep_canvas(x)
    in_maps = [
        {
            "xc": xcs[k],
            "wcur": wcur,
            "wprev": wprev,
            "w2t": w2t,
            "bias": bias,
            "scale": scale,
        }
        for k in range(N_CORES)
    ]
    res = run_bass_kernel_spmd(nc, in_maps, list(range(N_CORES)))
    out = np.empty((N_CORES * B, BO, H, T), np.float32)
    for k in range(N_CORES):
        hs = _decode_hs(res.results[k]["hs"])
        out[2 * k : 2 * k + 2] = hs
    return out


def _decode_hs(hs):
    """(B, T//CHUNK, BO, CHUNK*RS) raw ring dump -> (B, BO, H, T) = 2*SH."""
    hs = np.asarray(hs, np.float32).reshape(B, T // CHUNK, BO, CHUNK, RS)
    hs = hs[:, :, :, :, 1 : 1 + H]  # (B, nch, BO, CHUNK, H=row)
    hs = hs.transpose(0, 2, 4, 1, 3).reshape(B, BO, H, T)
    return hs

